# revision 1
# baseline (speedup 1.0000x reference)
"""Trainium2 Bass kernel for GQA attention (nn_Attention_50053548868012).

Math (reference):
  q = einsum('bsm,mrkh->brksh', x, wq);  k = x@wk;  v = x@wv        (per kv head)
  RoPE on q, k (k also scaled by H^-0.5), causal-masked softmax(q k^T),
  y = a @ v, out = einsum('brksh,rkhm->bsm', y, wo)

Sharding: tensor-parallel over the KV-head axis — core c owns kv head c
(its 4 query heads, wk/wv column slices, and a 512-wide slice of wo's
output dim). yT is AllGathered (chunked along seq) so each core computes
a 512-column slice of the output projection with the full 4096-dim
contraction. Host concatenates the 8 output slices.

Performance notes (measured on these axon trn2 cores):
 - matmuls cost ~0.65-0.9us each nearly independent of dtype; mm COUNT is
   what matters -> Z rows are summed on DVE/GPSIMD instead of PE.
 - DMA is ~60-80GB/s with 2KB per-partition lines but ~300+GB/s with 8KB
   lines -> all big tensors are host-packed so every large DMA moves
   [128, >=2048 f32] slabs; phase 1 runs m-blocks of 8 with SBUF
   accumulation so x slabs can be full 8KB-line reads.
 - float32r matmuls (same speed as bf16 here) with the whole operand
   chain declared f32r (walrus requires producers to round to f32r).
"""

import numpy as np

import concourse.bass as bass
import concourse.tile as tile
from concourse import bacc, mybir
from concourse.bass_utils import run_bass_kernel_spmd
from concourse.masks import make_identity

NCORES = 8
S = 2048
MD = 4096
H = 128
R = 4
KV = 8
PT = 128          # partition tile
SC = 512          # free-dim chunk
RH = R * H        # 512
MB = 8            # m-tiles per phase-1 block
SCALE = float(H) ** -0.5
NEG = -30000.0

f32 = mybir.dt.float32
f32r = mybir.dt.float32r


def build_bass(s=S, collective=True, phases=3, reps=1):
    nc = _emit(s, collective, phases, reps)
    nc.compile()
    return nc


def _emit(s, collective, phases, reps=1):
    assert s % SC == 0
    n_sc = s // SC          # seq chunks
    n_mt = MD // PT         # model-dim tiles (32)
    n_tt = s // PT          # seq tiles of 128
    tpc = SC // PT          # 128-tiles per chunk (4)
    n_blk = n_mt // MB      # phase-1 m-blocks (4)
    hh = H // 2

    nc = bacc.Bacc("TRN2", target_bir_lowering=False, debug=False,
                   num_devices=NCORES)

    # host-packed layouts: partition-major so big DMAs have 8KB+ lines
    xT = nc.dram_tensor("xT", [PT, n_mt, s], f32r, kind="ExternalInput").ap()
    wq = nc.dram_tensor("wq", [PT, n_mt, RH], f32r, kind="ExternalInput").ap()
    wk = nc.dram_tensor("wk", [PT, n_mt, H], f32r, kind="ExternalInput").ap()
    wv = nc.dram_tensor("wv", [PT, n_mt, H], f32r, kind="ExternalInput").ap()
    wo = nc.dram_tensor("wo", [PT, R, MD], f32r, kind="ExternalInput").ap()
    cosT = nc.dram_tensor("cosT", [H, s], f32, kind="ExternalInput").ap()
    sinT = nc.dram_tensor("sinT", [H, s], f32, kind="ExternalInput").ap()
    mask4 = nc.dram_tensor("mask4", [PT, tpc * SC], f32,
                           kind="ExternalInput").ap()
    outp = nc.dram_tensor("outp", [PT, n_tt, MD], f32,
                          kind="ExternalOutput").ap()

    with tile.TileContext(nc) as tc:
      for _rep in range(reps):
        with tc.tile_pool(name="const", bufs=1) as const_pool, \
             tc.tile_pool(name="dram", bufs=1, space="DRAM") as dram_pool:
            ones_f = const_pool.tile([PT, PT], f32)
            nc.gpsimd.memset(ones_f[:], 1.0)
            ones_sb = const_pool.tile([PT, PT], f32r)
            nc.scalar.copy(ones_sb[:], ones_f[:])


            ypersist_pool = tc.alloc_tile_pool(name="ypersist", bufs=1)
            yT_sb = ypersist_pool.tile([H, R, s], f32r)
            with tc.tile_pool(name="qkv", bufs=1) as qkv_pool:
                qT_sb = qkv_pool.tile([H, R, s], f32r)
                kT_sb = qkv_pool.tile([H, s], f32r)
                v_sb = qkv_pool.tile([PT, n_tt, H], f32r)

                # ---------- Phase 1: projections (m-blocked) + RoPE ----------
                with tc.tile_pool(name="ph1", bufs=1) as ph1_pool, \
                     tc.tile_pool(name="p1ps", bufs=1, space="PSUM") as p1_psum, \
                     tc.tile_pool(name="tpps", bufs=2, space="PSUM") as tp_psum:
                    w_ctx = tc.tile_pool(name="w1", bufs=1)
                    w_pool = w_ctx.__enter__()
                    x_ctx = tc.tile_pool(name="xslab", bufs=1)
                    x_pool = x_ctx.__enter__()
                    vT_sb = ph1_pool.tile([H, s], f32)
                    ident = ph1_pool.tile([PT, PT], f32)
                    make_identity(nc, ident[:])
                    # consts over SWDGE to keep HWDGE free for weight/x slabs
                    cos_sb = ph1_pool.tile([H, s], f32)
                    nc.gpsimd.dma_start(cos_sb[:], cosT)
                    sin_sb = ph1_pool.tile([H, s], f32)
                    nc.gpsimd.dma_start(sin_sb[:], sinT)

                    accs = {}
                    for j in range(R):
                        accs[j] = lambda ssl, j=j: qT_sb[:, j, ssl]
                    accs[R] = lambda ssl: kT_sb[:, ssl]
                    accs[R + 1] = lambda ssl: vT_sb[:, ssl]

                    for blk in range(n_blk):
                        wq_b = w_pool.tile([PT, MB, RH], f32r, tag="wqb")
                        nc.sync.dma_start(
                            wq_b[:], wq[:, blk * MB:(blk + 1) * MB, :])
                        wk_b = w_pool.tile([PT, MB, H], f32r, tag="wkb")
                        nc.sync.dma_start(
                            wk_b[:], wk[:, blk * MB:(blk + 1) * MB, :])
                        wv_b = w_pool.tile([PT, MB, H], f32r, tag="wvb")
                        nc.sync.dma_start(
                            wv_b[:], wv[:, blk * MB:(blk + 1) * MB, :])
                        xs = []
                        for ml in range(MB):
                            xsl = x_pool.tile([PT, s], f32r, tag=f"x{ml}",
                                              name="xsl")
                            nc.sync.dma_start(
                                xsl[:], xT[:, blk * MB + ml, :])
                            xs.append(xsl)
                        for sc_i in range(n_sc):
                            ssl = slice(sc_i * SC, (sc_i + 1) * SC)
                            ps6 = [p1_psum.tile([PT, SC], f32, tag=f"pa{u}",
                                                name=f"ps6_{u}")
                                   for u in range(R + 2)]
                            for ml in range(MB):
                                rx = xs[ml][:, ssl]
                                st = ml == 0
                                sp = ml == MB - 1
                                for j in range(R):
                                    nc.tensor.matmul(
                                        ps6[j][:],
                                        wq_b[:, ml, j * H:(j + 1) * H],
                                        rx, start=st, stop=sp)
                                nc.tensor.matmul(
                                    ps6[R][:], wk_b[:, ml, :], rx,
                                    start=st, stop=sp)
                                nc.tensor.matmul(
                                    ps6[R + 1][:], wv_b[:, ml, :], rx,
                                    start=st, stop=sp)
                            # spill/accumulate into SBUF (frees banks fast)
                            for u in range(R + 2):
                                acc = accs[u](ssl)
                                if blk == 0:
                                    nc.scalar.copy(acc, ps6[u][:])
                                else:
                                    nc.vector.tensor_add(
                                        acc, ps6[u][:], acc)

                    # RoPE in place on qT/kT. The half-rotation is done with
                    # SBUF->SBUF DMAs (engines can't mix SB base partitions),
                    # then three whole-tensor base-aligned DVE ops.
                    x_ctx.__exit__(None, None, None)
                    w_ctx.__exit__(None, None, None)
                    rope_ctx = tc.tile_pool(name="rope", bufs=1)
                    rope_pool = rope_ctx.__enter__()
                    qsw = rope_pool.tile([H, R, s], f32r, tag="qsw", bufs=1)
                    ksw = rope_pool.tile([H, s], f32r, tag="ksw", bufs=1)
                    nc.sync.dma_start(qsw[0:hh, :, :], qT_sb[hh:H, :, :])
                    nc.sync.dma_start(qsw[hh:H, :, :], qT_sb[0:hh, :, :])
                    nc.sync.dma_start(ksw[0:hh, :], kT_sb[hh:H, :])
                    nc.sync.dma_start(ksw[hh:H, :], kT_sb[0:hh, :])
                    sin_q = sin_sb[:, None, :].broadcast_to([H, R, s])
                    cos_q = cos_sb[:, None, :].broadcast_to([H, R, s])
                    nc.vector.tensor_mul(qsw[:], qsw[:], sin_q)
                    nc.vector.tensor_mul(qT_sb[:], qT_sb[:], cos_q)
                    nc.vector.tensor_add(qT_sb[:], qT_sb[:], qsw[:])
                    nc.vector.tensor_mul(ksw[:], ksw[:], sin_sb[:])
                    nc.vector.tensor_mul(kT_sb[:], kT_sb[:], cos_sb[:])
                    nc.vector.tensor_add(kT_sb[:], kT_sb[:], ksw[:])
                    for tt in range(n_tt):
                        ps_t = tp_psum.tile([PT, PT], f32, tag="tp",
                                            name="ps_t")
                        nc.tensor.transpose(
                            ps_t[:], vT_sb[:, tt * PT:(tt + 1) * PT],
                            ident[:])
                        nc.scalar.copy(v_sb[:, tt, :], ps_t[:])
                    rope_ctx.__exit__(None, None, None)

                if phases >= 2:
                    # ---------------- Phase 3 weights prefetch -------------------
                    with tc.tile_pool(name="w3", bufs=1) as w3_pool:
                        mask_sb = w3_pool.tile([PT, tpc, SC], f32)
                        nc.gpsimd.dma_start(
                            mask_sb[:], mask4.rearrange("p (j c) -> p j c", j=tpc))

                        # ---------------- Phase 2: attention ---------------------
                        with tc.tile_pool(name="epool", bufs=3) as e_pool, \
                             tc.tile_pool(name="zpool", bufs=2) as z_pool, \
                             tc.tile_pool(name="p2ps", bufs=2, space="PSUM") as p2_psum:
                            pending_fin = []
                            for c in range(n_sc):
                                T = (c + 1) * tpc
                                csl = slice(c * SC, (c + 1) * SC)
                                for j in range(R):
                                    ps_y = p2_psum.tile([H, SC], f32, tag="py")
                                    ps_z = p2_psum.tile([1, SC], f32, tag="pz",
                                                        bufs=1, name="ps_z")
                                    rq = qT_sb[:, j, csl]
                                    es = {}

                                    def qk_exp(p, rq=rq, T=T, es=es):
                                        # two score tiles in one 2-bank PSUM
                                        # tile; one mask add + one exp per pair
                                        t0 = 2 * p
                                        ps_s = p2_psum.tile([PT, 2 * SC], f32,
                                                            tag="ps", bufs=2,
                                                            name="ps_s")
                                        nc.tensor.matmul(
                                            ps_s[:, 0:SC],
                                            kT_sb[:, t0 * PT:(t0 + 1) * PT],
                                            rq, start=True, stop=True)
                                        nc.tensor.matmul(
                                            ps_s[:, SC:2 * SC],
                                            kT_sb[:, (t0 + 1) * PT:
                                                  (t0 + 2) * PT],
                                            rq, start=True, stop=True)
                                        jj = t0 - (T - tpc)
                                        if jj >= 0:
                                            nc.vector.tensor_add(
                                                ps_s[:].rearrange(
                                                    "q (a b) -> q a b", a=2),
                                                ps_s[:].rearrange(
                                                    "q (a b) -> q a b", a=2),
                                                mask_sb[:, jj:jj + 2, :])
                                        e_t = e_pool.tile([PT, 2 * SC], f32r,
                                                          tag="e", name="e_t")
                                        nc.scalar.activation(
                                            e_t[:], ps_s[:],
                                            mybir.ActivationFunctionType.Exp,
                                            scale=SCALE)
                                        es[p] = e_t

                                    P2 = T // 2
                                    qk_exp(0)
                                    # previous (c,j) normalization runs while
                                    # our QK prologue keeps the PE busy
                                    while pending_fin:
                                        pending_fin.pop(0)()
                                    for p in range(P2):
                                        if p + 1 < P2:
                                            qk_exp(p + 1)
                                        e_t = es.pop(p)
                                        for half in range(2):
                                            t = 2 * p + half
                                            esl = slice(half * SC,
                                                        (half + 1) * SC)
                                            nc.tensor.matmul(
                                                ps_y[:], v_sb[:, t, :],
                                                e_t[:, esl],
                                                start=(t == 0),
                                                stop=(t == T - 1))
                                            nc.tensor.matmul(
                                                ps_z[:], ones_sb[:, 0:1],
                                                e_t[:, esl],
                                                start=(t == 0),
                                                stop=(t == T - 1))

                                    def finalize(c=c, j=j, ps_y=ps_y, ps_z=ps_z,
                                                 csl=csl):
                                        rz = z_pool.tile([1, SC], f32r, tag="rz",
                                                         name="rz")
                                        with nc.allow_low_precision(
                                                reason="f32r is full-width"):
                                            nc.vector.reciprocal(rz[:], ps_z[:])
                                        ps_b = p2_psum.tile([PT, SC], f32,
                                                            tag="pb", bufs=1,
                                                            name="ps_b")
                                        nc.tensor.matmul(
                                            ps_b[:], ones_sb[0:1, :], rz[:],
                                            start=True, stop=True)
                                        b_sb = z_pool.tile([PT, SC], f32,
                                                           tag="bsb", name="b_sb")
                                        nc.scalar.copy(b_sb[:], ps_b[:])
                                        nc.vector.tensor_mul(
                                            yT_sb[:, j, csl], ps_y[:], b_sb[:])

                                    pending_fin.append(finalize)
                            while pending_fin:
                                pending_fin.pop(0)()

            # -------- Phase 3: local-rh output projection (host sums) -------
            if phases >= 3:
              with tc.tile_pool(name="w3b", bufs=1) as w3b_pool, \
                 tc.tile_pool(name="osb", bufs=2) as o_pool, \
                 tc.tile_pool(name="p3ps", bufs=1, space="PSUM") as p3_psum:
                wo_sb = w3b_pool.tile([PT, R, MD], f32r)
                for rl in range(R):
                    nc.sync.dma_start(wo_sb[:, rl, :], wo[:, rl, :])
                n_mc = MD // RH
                for st in range(n_tt):
                    o_acc = o_pool.tile([PT, MD], f32, tag="oacc",
                                        name="o_acc")
                    for mc in range(n_mc):
                        ps_o = p3_psum.tile([PT, RH], f32, tag=f"o{mc % 4}",
                                            bufs=2, name="ps_o")
                        for rl in range(R):
                            nc.tensor.matmul(
                                ps_o[:],
                                yT_sb[:, rl, st * PT:(st + 1) * PT],
                                wo_sb[:, rl, mc * RH:(mc + 1) * RH],
                                start=(rl == 0), stop=(rl == R - 1))
                        nc.scalar.copy(
                            o_acc[:, mc * RH:(mc + 1) * RH], ps_o[:])
                    nc.sync.dma_start(outp[:, st, :], o_acc[:])
            ypersist_pool.release()
    return nc


def make_mask4():
    """mask4[:, 512j:512(j+1)][ti, sj] = 0 if 128j+ti <= sj else NEG."""
    tpc = SC // PT
    m = np.full((PT, tpc * SC), NEG, dtype=np.float32)
    for j in range(tpc):
        ti = np.arange(PT)[:, None]
        sj = np.arange(SC)[None, :]
        m[:, j * SC:(j + 1) * SC] = np.where(128 * j + ti <= sj, 0.0, NEG)
    return m


def _pack_pm(a):
    """[n_mt*128, C] -> [128, n_mt, C] (partition-major for 8KB DMA lines)."""
    n_mt = a.shape[0] // PT
    return np.ascontiguousarray(
        a.reshape(n_mt, PT, a.shape[1]).transpose(1, 0, 2))


def shard_inputs(x, wq, wk, wv, wo, mask, sin, cos, s=S):
    """Build the 8 per-core input maps from the full problem inputs."""
    del mask  # causality is hardcoded (mask4 tiles)
    xTp = _pack_pm(np.ascontiguousarray(
        np.asarray(x, dtype=np.float32).reshape(s, MD).T))
    cosT = np.ascontiguousarray(np.asarray(cos, dtype=np.float32).T)
    sinT = np.ascontiguousarray(np.asarray(sin, dtype=np.float32).T)
    sign = np.concatenate(
        [-np.ones((H // 2, 1)), np.ones((H // 2, 1))]).astype(np.float32)
    sinTs = np.ascontiguousarray(sinT * sign)
    wo = np.asarray(wo, dtype=np.float32)
    mask4 = make_mask4()
    wq = np.asarray(wq, dtype=np.float32)
    wk = np.asarray(wk, dtype=np.float32)
    wv = np.asarray(wv, dtype=np.float32)
    in_maps = []
    for c in range(NCORES):
        in_maps.append({
            "xT": xTp,
            "wq": _pack_pm(np.ascontiguousarray(
                wq[:, :, c, :].reshape(MD, RH))),
            "wk": _pack_pm(np.ascontiguousarray(wk[:, c, :])),
            "wv": _pack_pm(np.ascontiguousarray(wv[:, c, :])),
            "wo": _pack_pm(np.ascontiguousarray(
                wo[:, c, :, :].reshape(RH, MD))),
            "cosT": cosT,
            "sinT": sinTs,
            "mask4": mask4,
        })
    return in_maps


def unpack_out(outp_arr, s=S):
    """[128, s/128, MD] -> [s, MD]."""
    return np.ascontiguousarray(
        np.asarray(outp_arr).reshape(PT, s // PT, MD).transpose(
            1, 0, 2).reshape(s, MD))


_NC_CACHE = {}


def kernel(x, wq, wk, wv, wo, mask, sin, cos):
    s = x.shape[1]
    if s not in _NC_CACHE:
        _NC_CACHE[s] = build_bass(s)
    nc = _NC_CACHE[s]
    in_maps = shard_inputs(x, wq, wk, wv, wo, mask, sin, cos, s=s)
    res = run_bass_kernel_spmd(nc, in_maps, list(range(NCORES)))
    out = unpack_out(res.results[0]["outp"], s)
    for c in range(1, NCORES):
        out = out + unpack_out(res.results[c]["outp"], s)
    return out.reshape(1, s, MD).astype(np.float32)



# revision 2
# speedup vs baseline: 1.0163x; 1.0163x over previous
"""Trainium2 Bass kernel v2 for GQA attention (nn_Attention_50053548868012).

Deltas vs v1 (kernel.py):
 - all matmul operands bf16 (same PE rate as f32r on this rig, but halves
   every DMA and SBUF footprint, and enables FWL weight loads)
 - phase 1: MB=16 two-block structure, full 16-deep PSUM accumulation per
   block (one SBUF add instead of three), per-chunk RoPE emitted inline so
   DVE/DMA rope work overlaps the next chunk's matmuls (kills the ~45us
   PE stall between phases 1 and 2)
 - causal mask generated on device (gpsimd affine_select), no mask input
 - output written bf16 (host upcasts and sums the 8 partials)
"""

import numpy as np
import ml_dtypes

import concourse.bass as bass
import concourse.tile as tile
from concourse import bacc, mybir
from concourse.bass_utils import run_bass_kernel_spmd
from concourse.masks import make_identity

NCORES = 8
S = 2048
MD = 4096
H = 128
R = 4
KV = 8
PT = 128          # partition tile
SC = 512          # free-dim chunk
RH = R * H        # 512
MB = 16           # m-tiles per phase-1 block
SCALE = float(H) ** -0.5
NEG = -30000.0

f32 = mybir.dt.float32
f32r = mybir.dt.float32r
bf16 = mybir.dt.bfloat16
BFNP = ml_dtypes.bfloat16


def build_bass(s=S, collective=True, phases=3, reps=1):
    nc = _emit(s, collective, phases, reps)
    nc.compile()
    return nc


def _emit(s, collective, phases, reps=1):
    assert s % SC == 0
    n_sc = s // SC          # seq chunks (4)
    n_mt = MD // PT         # model-dim tiles (32)
    n_tt = s // PT          # seq tiles of 128 (16)
    tpc = SC // PT          # 128-tiles per chunk (4)
    n_blk = n_mt // MB      # phase-1 m-blocks (2)
    hh = H // 2

    nc = bacc.Bacc("TRN2", target_bir_lowering=False, debug=False,
                   num_devices=NCORES)

    xT = nc.dram_tensor("xT", [PT, n_mt, s], bf16, kind="ExternalInput").ap()
    wq = nc.dram_tensor("wq", [PT, n_mt, RH], bf16, kind="ExternalInput").ap()
    wk = nc.dram_tensor("wk", [PT, n_mt, H], bf16, kind="ExternalInput").ap()
    wv = nc.dram_tensor("wv", [PT, n_mt, H], bf16, kind="ExternalInput").ap()
    wo = nc.dram_tensor("wo", [PT, R, MD], bf16, kind="ExternalInput").ap()
    cosT = nc.dram_tensor("cosT", [H, s], bf16, kind="ExternalInput").ap()
    sinT = nc.dram_tensor("sinT", [H, s], bf16, kind="ExternalInput").ap()
    outp = nc.dram_tensor("outp", [PT, n_tt, MD], bf16,
                          kind="ExternalOutput").ap()

    with tile.TileContext(nc) as tc:
      for _rep in range(reps):
        with tc.tile_pool(name="const", bufs=1) as const_pool, \
             tc.tile_pool(name="dram", bufs=1, space="DRAM") as dram_pool:
            ones_f = const_pool.tile([PT, PT], f32)
            nc.gpsimd.memset(ones_f[:], 1.0)
            ones_b = const_pool.tile([PT, PT], bf16)
            nc.scalar.copy(ones_b[:], ones_f[:])
            ones_r = const_pool.tile([PT, PT], f32r)
            nc.scalar.copy(ones_r[:], ones_f[:])
            ident = const_pool.tile([PT, PT], bf16)
            make_identity(nc, ident[:])
            # causal mask for the diagonal chunk: mask[p, j, q] = 0 where
            # 128*j + p <= q else NEG  (j = k-tile within chunk, q in chunk)
            mask_sb = const_pool.tile([PT, tpc, SC], bf16)
            nc.gpsimd.memset(mask_sb[:], 0.0)
            nc.gpsimd.affine_select(
                out=mask_sb[:], in_=mask_sb[:],
                compare_op=mybir.AluOpType.is_ge, fill=NEG, base=0,
                pattern=[[-PT, tpc], [1, SC]], channel_multiplier=-1)
            cos_sb = const_pool.tile([H, s], bf16)
            nc.gpsimd.dma_start(cos_sb[:], cosT)
            sin_sb = const_pool.tile([H, s], bf16)
            nc.gpsimd.dma_start(sin_sb[:], sinT)

            persist_ctx = tc.tile_pool(name="persist", bufs=1)
            persist = persist_ctx.__enter__()
            qT_sb = persist.tile([H, R, s], bf16)
            kT_sb = persist.tile([H, s], bf16)
            v_sb = persist.tile([PT, n_tt, H], bf16)
            yT_sb = persist.tile([H, R, s], bf16)

            # ---------- Phase 1: projections (2 m-blocks) + inline RoPE ----
            with tc.tile_pool(name="ph1", bufs=1) as ph1_pool, \
                 tc.tile_pool(name="w1", bufs=2) as w_pool, \
                 tc.tile_pool(name="xslab", bufs=1) as x_pool, \
                 tc.tile_pool(name="rope", bufs=2) as rope_pool, \
                 tc.tile_pool(name="p1ps", bufs=1, space="PSUM") as p1_psum, \
                 tc.tile_pool(name="tpps", bufs=2, space="PSUM") as tp_psum:
                # per-chunk pre-rope accumulators: separate tiles so chunk
                # c+1's spills don't serialize behind chunk c's rope
                # (tile-granular dependency tracking)
                qacc = [ph1_pool.tile([H, R, SC], bf16, tag=f"qa{c}",
                                      name=f"qa{c}") for c in range(n_sc)]
                kacc = [ph1_pool.tile([H, SC], bf16, tag=f"ka{c}",
                                      name=f"ka{c}") for c in range(n_sc)]
                vacc = [ph1_pool.tile([H, SC], bf16, tag=f"va{c}",
                                      name=f"va{c}") for c in range(n_sc)]
                pending_rope = []

                for blk in range(n_blk):
                    # k/v weights + first x slab first so the PE can start
                    # ~2us in; wq (2MB) streams under the first k/v matmuls
                    xs = []
                    x0 = x_pool.tile([PT, s], bf16, tag="x0", name="x0")
                    nc.sync.dma_start(x0[:], xT[:, blk * MB, :])
                    xs.append(x0)
                    wk_b = w_pool.tile([PT, MB, H], bf16, tag="wkb",
                                       name="wk_b")
                    nc.sync.dma_start(
                        wk_b[:], wk[:, blk * MB:(blk + 1) * MB, :])
                    wv_b = w_pool.tile([PT, MB, H], bf16, tag="wvb",
                                       name="wv_b")
                    nc.sync.dma_start(
                        wv_b[:], wv[:, blk * MB:(blk + 1) * MB, :])
                    wq_b = w_pool.tile([PT, MB, RH], bf16, tag="wqb",
                                       name="wq_b")
                    nc.sync.dma_start(
                        wq_b[:], wq[:, blk * MB:(blk + 1) * MB, :])
                    for ml in range(1, MB):
                        xsl = x_pool.tile([PT, s], bf16, tag=f"x{ml}",
                                          name="xsl")
                        nc.sync.dma_start(xsl[:], xT[:, blk * MB + ml, :])
                        xs.append(xsl)
                    for sc_i in range(n_sc):
                        ssl = slice(sc_i * SC, (sc_i + 1) * SC)
                        ps6 = [p1_psum.tile([PT, SC], f32, tag=f"pa{u}",
                                            name=f"ps6_{u}")
                               for u in range(R + 2)]
                        for ml in range(MB):
                            rx = xs[ml][:, ssl]
                            st = ml == 0
                            sp = ml == MB - 1
                            nc.tensor.matmul(
                                ps6[R][:], wk_b[:, ml, :], rx,
                                start=st, stop=sp)
                            nc.tensor.matmul(
                                ps6[R + 1][:], wv_b[:, ml, :], rx,
                                start=st, stop=sp)
                            for j in range(R):
                                nc.tensor.matmul(
                                    ps6[j][:],
                                    wq_b[:, ml, j * H:(j + 1) * H],
                                    rx, start=st, stop=sp)
                        accs = ([qacc[sc_i][:, j, :] for j in range(R)]
                                + [kacc[sc_i][:], vacc[sc_i][:]])
                        for u in range(R + 2):
                            if blk == 0:
                                nc.scalar.copy(accs[u], ps6[u][:])
                            else:
                                nc.vector.tensor_add(
                                    accs[u], ps6[u][:], accs[u])
                        if blk == n_blk - 1:
                            # swap-DMAs issue now (Pool/SWDGE, overlapped);
                            # the DVE rope math is DEFERRED one chunk so it
                            # queues BEHIND the next chunk's spill-adds in
                            # the DVE FIFO (bank release stays prompt)
                            qsw = rope_pool.tile([H, R, SC], bf16, tag="qsw",
                                                 name="qsw")
                            nc.gpsimd.dma_start(
                                qsw[0:hh, :, :], qacc[sc_i][hh:H, :, :])
                            nc.gpsimd.dma_start(
                                qsw[hh:H, :, :], qacc[sc_i][0:hh, :, :])
                            ksw = rope_pool.tile([H, SC], bf16, tag="ksw",
                                                 name="ksw")
                            nc.gpsimd.dma_start(
                                ksw[0:hh, :], kacc[sc_i][hh:H, :])
                            nc.gpsimd.dma_start(
                                ksw[hh:H, :], kacc[sc_i][0:hh, :])

                            def rope_math(sc_i=sc_i, ssl=ssl, qsw=qsw,
                                          ksw=ksw):
                                # per-head ops: no broadcast APs (keeps DVE
                                # fast mode), finer FIFO interleave
                                for j in range(R):
                                    nc.vector.tensor_mul(
                                        qsw[:, j, :], qsw[:, j, :],
                                        sin_sb[:, ssl])
                                    nc.vector.tensor_mul(
                                        qT_sb[:, j, ssl],
                                        qacc[sc_i][:, j, :],
                                        cos_sb[:, ssl])
                                    nc.vector.tensor_add(
                                        qT_sb[:, j, ssl], qT_sb[:, j, ssl],
                                        qsw[:, j, :])
                                nc.vector.tensor_mul(
                                    ksw[:], ksw[:], sin_sb[:, ssl])
                                nc.vector.tensor_mul(
                                    kT_sb[:, ssl], kacc[sc_i][:],
                                    cos_sb[:, ssl])
                                nc.vector.tensor_add(
                                    kT_sb[:, ssl], kT_sb[:, ssl], ksw[:])
                                for tl in range(tpc):
                                    tt = sc_i * tpc + tl
                                    ps_t = tp_psum.tile([PT, PT], bf16,
                                                        tag="tp",
                                                        name="ps_t")
                                    nc.tensor.transpose(
                                        ps_t[:],
                                        vacc[sc_i][:, tl * PT:(tl + 1) * PT],
                                        ident[:])
                                    nc.scalar.copy(v_sb[:, tt, :], ps_t[:])

                            pending_rope.append(rope_math)
                            if len(pending_rope) > 1:
                                pending_rope.pop(0)()
                while pending_rope:
                    pending_rope.pop(0)()

            if phases >= 2:
              # wo pool spans phases 2+3 (prefetch overlaps attention)
              with tc.tile_pool(name="w3", bufs=1) as w3_pool:
                wo_sb = w3_pool.tile([PT, R, MD], bf16)
                for rl in range(R):
                    nc.sync.dma_start(wo_sb[:, rl, :], wo[:, rl, :])

                # ------- Phase 2: attention (+ interleaved phase 3) -------
                n_mc = MD // RH
                with tc.tile_pool(name="epool", bufs=6) as e_pool, \
                     tc.tile_pool(name="zpool", bufs=2) as z_pool, \
                     tc.tile_pool(name="osb", bufs=3) as o_pool, \
                     tc.tile_pool(name="p2ps", bufs=2, space="PSUM") as p2_psum:
                    pending_fin = []
                    pending_ph3 = []

                    def emit_ph3(c):
                        # output projection for the 4 seq-tiles of chunk c
                        # (dense, always-ready PE filler between chunks)
                        for tl in range(tpc):
                            st = c * tpc + tl
                            o_acc = o_pool.tile([PT, MD], bf16, tag="oacc",
                                                name="o_acc")
                            for mc in range(n_mc):
                                ps_o = p2_psum.tile(
                                    [PT, RH], f32, tag="po", bufs=2,
                                    name="ps_o")
                                for rl in range(R):
                                    nc.tensor.matmul(
                                        ps_o[:],
                                        yT_sb[:, rl,
                                              st * PT:(st + 1) * PT],
                                        wo_sb[:, rl,
                                              mc * RH:(mc + 1) * RH],
                                        start=(rl == 0),
                                        stop=(rl == R - 1))
                                if mc % 2 == 0:
                                    nc.scalar.copy(
                                        o_acc[:, mc * RH:(mc + 1) * RH],
                                        ps_o[:])
                                else:
                                    nc.vector.tensor_copy(
                                        o_acc[:, mc * RH:(mc + 1) * RH],
                                        ps_o[:])
                            nc.sync.dma_start(outp[:, st, :], o_acc[:])
                    for c in range(n_sc):
                        T = (c + 1) * tpc
                        csl = slice(c * SC, (c + 1) * SC)
                        for j in range(R):
                            ps_y = p2_psum.tile([H, SC], f32, tag="py",
                                                bufs=1, name="ps_y")
                            ps_z = p2_psum.tile([1, SC], f32, tag="pz",
                                                bufs=1, name="ps_z")
                            rq = qT_sb[:, j, csl]
                            es = {}

                            def qk_exp(p, rq=rq, T=T, es=es):
                                t0 = 2 * p
                                ps_s = p2_psum.tile([PT, 2 * SC], f32,
                                                    tag="ps", bufs=2,
                                                    name="ps_s")
                                nc.tensor.matmul(
                                    ps_s[:, 0:SC],
                                    kT_sb[:, t0 * PT:(t0 + 1) * PT],
                                    rq, start=True, stop=True)
                                nc.tensor.matmul(
                                    ps_s[:, SC:2 * SC],
                                    kT_sb[:, (t0 + 1) * PT:(t0 + 2) * PT],
                                    rq, start=True, stop=True)
                                e_t = e_pool.tile([PT, 2 * SC], bf16,
                                                  tag="e", name="e_t")
                                nc.scalar.activation(
                                    e_t[:], ps_s[:],
                                    mybir.ActivationFunctionType.Exp,
                                    scale=SCALE)
                                dt0 = t0 - (T - tpc)
                                if dt0 >= 0:
                                    # zero the causally-masked region on the
                                    # idle Pool engine: keep where
                                    # q - p - 128*(dt0 + a) >= 0
                                    ev = e_t[:].rearrange(
                                        "k (a b) -> k a b", a=2)
                                    nc.gpsimd.affine_select(
                                        out=ev, in_=ev,
                                        compare_op=mybir.AluOpType.is_ge,
                                        fill=0.0, base=-PT * dt0,
                                        pattern=[[-PT, 2], [1, SC]],
                                        channel_multiplier=-1)
                                es[p] = e_t

                            P2 = T // 2
                            # diagonal pair first: its DVE mask-add + exp
                            # latency hides under the previous head's AV
                            # tail instead of stalling this head's tail
                            order = [P2 - 1, P2 - 2] + list(range(P2 - 2))
                            qk_exp(order[0])
                            while pending_fin:
                                pending_fin.pop(0)()
                            # z matmuls deferred one pair behind the AV
                            # matmuls: consecutive `ones` stationary loads
                            # (no weight-buffer thrash) and always-ready
                            # PE filler during exp waits
                            ny = 0
                            nz = 0
                            zq = []
                            for oi, p in enumerate(order):
                                if oi + 1 < P2:
                                    qk_exp(order[oi + 1])
                                e_t = es.pop(p)
                                for half in range(2):
                                    t = 2 * p + half
                                    esl = slice(half * SC, (half + 1) * SC)
                                    nc.tensor.matmul(
                                        ps_y[:], v_sb[:, t, :],
                                        e_t[:, esl],
                                        start=(ny == 0),
                                        stop=(ny == T - 1))
                                    ny += 1
                                    zq.append(e_t[:, esl])
                                if oi > 0:
                                    while len(zq) > 2:
                                        ez = zq.pop(0)
                                        nc.tensor.matmul(
                                            ps_z[:], ones_b[:, 0:1], ez,
                                            start=(nz == 0),
                                            stop=(nz == T - 1))
                                        nz += 1
                            while zq:
                                ez = zq.pop(0)
                                nc.tensor.matmul(
                                    ps_z[:], ones_b[:, 0:1], ez,
                                    start=(nz == 0), stop=(nz == T - 1))
                                nz += 1

                            def finalize(c=c, j=j, ps_y=ps_y, ps_z=ps_z,
                                         csl=csl):
                                rz = z_pool.tile([1, SC], f32, tag="rz",
                                                 name="rz")
                                with nc.allow_low_precision(
                                        reason="full-width recip"):
                                    nc.vector.reciprocal(rz[:], ps_z[:])
                                # broadcast 1/z across partitions on the
                                # (idle) Pool engine instead of PE+ACT
                                b_sb = z_pool.tile([PT, SC], f32,
                                                   tag="bsb", name="b_sb")
                                nc.gpsimd.partition_broadcast(
                                    b_sb[:], rz[:])
                                nc.vector.tensor_mul(
                                    yT_sb[:, j, csl], ps_y[:], b_sb[:])

                            pending_fin.append(finalize)
                        if phases >= 3:
                            pending_ph3.append(c)
                            if len(pending_ph3) > 1:
                                emit_ph3(pending_ph3.pop(0))
                    while pending_fin:
                        pending_fin.pop(0)()
                    while pending_ph3:
                        emit_ph3(pending_ph3.pop(0))

            persist_ctx.__exit__(None, None, None)
    return nc


def _pack_pm(a):
    """[n_mt*128, C] -> [128, n_mt, C] partition-major, bf16."""
    n_mt = a.shape[0] // PT
    return np.ascontiguousarray(
        a.reshape(n_mt, PT, a.shape[1]).transpose(1, 0, 2)).astype(BFNP)


def shard_inputs(x, wq, wk, wv, wo, mask, sin, cos, s=S):
    del mask  # causality generated on device
    xTp = _pack_pm(np.ascontiguousarray(
        np.asarray(x, dtype=np.float32).reshape(s, MD).T))
    cosT = np.ascontiguousarray(
        np.asarray(cos, dtype=np.float32).T).astype(BFNP)
    sign = np.concatenate(
        [-np.ones((H // 2, 1)), np.ones((H // 2, 1))]).astype(np.float32)
    sinTs = np.ascontiguousarray(
        np.asarray(sin, dtype=np.float32).T * sign).astype(BFNP)
    wo = np.asarray(wo, dtype=np.float32)
    wq = np.asarray(wq, dtype=np.float32)
    wk = np.asarray(wk, dtype=np.float32)
    wv = np.asarray(wv, dtype=np.float32)
    in_maps = []
    for c in range(NCORES):
        in_maps.append({
            "xT": xTp,
            "wq": _pack_pm(np.ascontiguousarray(
                wq[:, :, c, :].reshape(MD, RH))),
            "wk": _pack_pm(np.ascontiguousarray(wk[:, c, :])),
            "wv": _pack_pm(np.ascontiguousarray(wv[:, c, :])),
            "wo": _pack_pm(np.ascontiguousarray(
                wo[:, c, :, :].reshape(RH, MD))),
            "cosT": cosT,
            "sinT": sinTs,
        })
    return in_maps


def unpack_out(outp_arr, s=S):
    """[128, s/128, MD] bf16 -> [s, MD] f32."""
    return np.ascontiguousarray(
        np.asarray(outp_arr).astype(np.float32).reshape(
            PT, s // PT, MD).transpose(1, 0, 2).reshape(s, MD))


_NC_CACHE = {}


def kernel(x, wq, wk, wv, wo, mask, sin, cos):
    s = x.shape[1]
    if s not in _NC_CACHE:
        _NC_CACHE[s] = build_bass(s)
    nc = _NC_CACHE[s]
    in_maps = shard_inputs(x, wq, wk, wv, wo, mask, sin, cos, s=s)
    res = run_bass_kernel_spmd(nc, in_maps, list(range(NCORES)))
    out = unpack_out(res.results[0]["outp"], s)
    for c in range(1, NCORES):
        out = out + unpack_out(res.results[c]["outp"], s)
    return out.reshape(1, s, MD).astype(np.float32)
